# revision 1
# baseline (speedup 1.0000x reference)
"""Trainium2 Bass kernel for nn_FilmLayerNetwork.

Sharding: one NeuronCore per SMAB processor (NPROC = 8 = n_cores).
Each core computes its processor's full 512-map output slice.

Performance notes (from NTFF traces):
- fp32 matmuls run as LOW/HIGH pass pairs with a ~300ns-per-instruction
  floor, so matmul COUNT dominates PE time. Stage 0 computes [Qk|Kk|Vv]
  with 4 matmuls (fused 288-wide rhs, P as the 8-col stationary) and the
  per-head transposed views are recovered with one DVE 32x32 block
  transpose; independent matmul groups (alphaT, qT) are hand-interleaved
  into the serial attention chain's PE gaps (the PE runs its queue in
  order, so emission order is placement).
- HWDGE services all queued transfers serially in global issue order, so
  all input DMAs go on ONE queue in consumption order, with w1/wa split
  into chunks so dependent matmuls start as each chunk lands.
- de = (De*gate).sum(-1) runs on the vector engine; the FiLM tail runs
  in a (128,4) layout (128-partition vector ops are ~150ns vs ~600-3300ns
  for 1-partition ops); sigmoid is computed as 1/(1+exp(-x)) to keep a
  single scalar-engine activation table resident (a swap costs 1.3us).
- All matmuls sit at PE tile position (0,0): no partition-offset operands
  (cross-tile PSUM-bank hazards) and no PE transposes.
"""

import numpy as np

NM, ZG, HID, SEQ = 512, 512, 96, 8
H1, H2, NPROC, NB = 3, 16, 8, 2
SCL = float(1.0 / np.sqrt(96.0))

# b96 column layout
C_F1, C_WQ2, C_WK2, C_WV2, C_F2, C_WO, C_E, C_BQR = (
    0, 96, 192, 288, 384, 480, 992, 1008)
B96_COLS = 1009
# px column layout: P chunks | xT | baT | regsT | offsT | De_pm | gate128
# (the FiLM-tail columns ride in the px DMA: every extra small transfer
# costs ~0.7us of serial HWDGE queue time)
C_BA, C_RG, C_OF, C_DE, C_G = 36, 40, 44, 45, 77
PX_COLS = 85

_CACHE = {}


def _build_nc():
    import concourse.bass as bass
    import concourse.bacc as bacc
    import concourse.tile as tile
    import concourse.mybir as mybir

    f32 = mybir.dt.float32
    AX = mybir.AxisListType
    ALU = mybir.AluOpType
    ACT = mybir.ActivationFunctionType

    nc = bacc.Bacc("TRN2", target_bir_lowering=False, debug=False, num_devices=NPROC)

    bf16 = mybir.dt.bfloat16
    d_px = nc.dram_tensor("px", [128, PX_COLS], f32, kind="ExternalInput").ap()
    d_w1 = nc.dram_tensor("w1", [128, 1152], f32, kind="ExternalInput").ap()
    d_b96 = nc.dram_tensor("b96", [96, B96_COLS], f32, kind="ExternalInput").ap()
    d_wqr = nc.dram_tensor("wqr", [128, 384], f32, kind="ExternalInput").ap()
    d_b16 = nc.dram_tensor("b16", [16, 96], f32, kind="ExternalInput").ap()
    d_wa = nc.dram_tensor("wa", [128, 2048], bf16, kind="ExternalInput").ap()
    d_out = nc.dram_tensor("out", [128, 4], f32, kind="ExternalOutput").ap()

    with tile.TileContext(nc) as tc, \
         tc.tile_pool(name="sb", bufs=1) as sb, \
         tc.tile_pool(name="ps", bufs=8, space="PSUM") as ps:

        def sbt(shape, tag, dt=f32):
            return sb.tile(shape, dt, tag=tag, name=tag)

        def pst(shape, tag):
            return ps.tile(shape, f32, tag="ps_shared", name=tag)

        # ---- input DMAs: one HWDGE queue, consumption order, big tensors
        # chunked so consumers start early ----
        sb_px = sbt([128, PX_COLS], "sb_px")
        nc.scalar.dma_start(out=sb_px[:], in_=d_px[:])
        sb_w1k = []
        for k in range(4):
            t = sbt([128, 288], f"sb_w1k{k}")
            nc.scalar.dma_start(out=t[:], in_=d_w1[:, 288 * k:288 * k + 288])
            sb_w1k.append(t)
        # wa as ONE bf16 transfer (half the bytes of fp32, and bf16 alpha
        # matmuls are single-pass instead of LOW/HIGH pairs; the resulting
        # output error is ~2e-6 absolute because alpha feeds mix*regs with
        # regs ~1e-3). It lands right after w1 so the 16 alphaT matmuls can
        # fill PE gaps in the serial attention chain.
        sb_wa = sbt([128, 2048], "sb_wa", bf16)
        nc.scalar.dma_start(out=sb_wa[:], in_=d_wa[:])
        sb_wqr = sbt([128, 384], "sb_wqr")
        nc.scalar.dma_start(out=sb_wqr[:], in_=d_wqr[:])
        sb_16 = sbt([16, 96], "sb_16")
        nc.scalar.dma_start(out=sb_16[:], in_=d_b16[:])
        sb_96 = sbt([96, B96_COLS], "sb_96")
        nc.scalar.dma_start(out=sb_96[:], in_=d_b96[:])

        sb_t = sb_px   # FiLM-tail columns live inside the px transfer
        P_blk = lambda k: sb_px[:, 8 * k:8 * k + 8]
        xT_blk = lambda k: sb_px[:, 32 + k:33 + k]

        # bf16 copy of xT for the bf16 alpha matmuls (both matmul operands
        # must share the fp32-ness)
        sb_xb = sbt([128, 4], "sb_xb", bf16)
        nc.vector.tensor_copy(sb_xb[:], sb_px[:, 32:36])

        # ---- stage 0: [Qk | Kk | Vv] (8, 288) in 4 fused matmuls ----
        ps_qkv = pst([8, 288], "ps_qkv")
        for k in range(4):
            nc.tensor.matmul(ps_qkv[:], P_blk(k), sb_w1k[k][:],
                             start=(k == 0), stop=(k == 3))

        sb_qkv = sbt([8, 288], "sb_qkv")
        nc.scalar.copy(sb_qkv[:], ps_qkv[:])
        Qk = sb_qkv[:, 0:96]
        Kk = sb_qkv[:, 96:192]
        Vv = sb_qkv[:, 192:288]

        # per-head transposed views of Qk and Kk via one DVE block transpose
        sb_t32 = sbt([32, 192], "sb_t32")
        nc.vector.memset(sb_t32[:], 0.0)
        nc.scalar.copy(sb_t32[0:8, 0:96], Qk)
        nc.scalar.copy(sb_t32[0:8, 96:192], Kk)
        sb_tT = sbt([32, 192], "sb_tT")
        nc.vector.transpose(sb_tT[:], sb_t32[:])

        def QkT_h(h):
            return sb_tT[0:32, 32 * h:32 * h + 8]

        def KkT_h(h):
            return sb_tT[0:32, 96 + 32 * h:96 + 32 * h + 8]

        # MHA1 scores, per head, all at PE tile position (0,0)
        ps_s = pst([8, 24], "ps_s")
        for h in range(3):
            nc.tensor.matmul(ps_s[:, 8 * h:8 * h + 8], QkT_h(h), KkT_h(h))

        # QkT (96,8) contiguous for the attention residual
        sb_qkT = sbt([96, 8], "sb_qkT")
        for h in range(3):
            nc.scalar.copy(sb_qkT[32 * h:32 * h + 32, :], QkT_h(h))

        # softmax (magnitudes are small; max-subtraction unnecessary),
        # normalized A written directly into the 32x32-block layout
        sb_a32 = sbt([32, 96], "sb_a32")
        nc.vector.memset(sb_a32[:], 0.0)
        a32v = sb_a32[0:8, :].rearrange("p (h x) -> p h x", h=3)[:, :, 0:8]
        nc.scalar.activation(a32v, ps_s[:].rearrange("p (h x) -> p h x", h=3),
                             ACT.Exp, scale=SCL)
        sb_sums = sbt([8, 3], "sb_sums")
        nc.vector.tensor_reduce(sb_sums[:], a32v, AX.X, ALU.add)
        sb_rec = sbt([8, 3], "sb_rec")
        nc.vector.reciprocal(sb_rec[:], sb_sums[:])
        rec_ap = sb_rec[:]
        rec_bc = bass.AP(tensor=rec_ap.tensor, offset=rec_ap.offset,
                         ap=[rec_ap.ap[0], rec_ap.ap[1], [0, 8]])
        nc.vector.tensor_tensor(a32v, a32v, rec_bc, ALU.mult)
        sb_aT32 = sbt([32, 96], "sb_aT32")
        nc.vector.transpose(sb_aT32[:], sb_a32[:])

        def A_T(h):
            return sb_aT32[0:8, 32 * h:32 * h + 8]

        sb_vm = sbt([8, 288], "sb_vm")
        nc.vector.memset(sb_vm[:], 0.0)
        for h in range(3):
            nc.scalar.copy(sb_vm[:, 128 * h:128 * h + 32],
                           Vv[:, 32 * h:32 * h + 32])

        # alphaT (128,4): 16 (k,m) chunk matmuls, interleaved into the
        # chain's PE gaps; each group m only needs wa chunk m
        ps_al = pst([128, 4], "ps_al")

        def alpha_mms(ms):
            for m in ms:
                for k in range(4):
                    nc.tensor.matmul(
                        ps_al[:, m:m + 1],
                        sb_wa[:, 512 * k + 128 * m:512 * k + 128 * m + 128],
                        sb_xb[:, k:k + 1], start=(k == 0), stop=(k == 3))

        alpha_mms([0])

        # qT: contraction chunks over Wqr (wqr lands right after wa)
        ps_qT = pst([96, 1], "ps_qT")
        for k in range(4):
            nc.tensor.matmul(ps_qT[:], sb_wqr[:, 96 * k:96 * k + 96], xT_blk(k),
                             start=(k == 0), stop=(k == 3))

        # O^T = sum_h maskedV_h^T @ A_T_h at tile position (0,0)
        ps_oT = pst([96, 8], "ps_oT")
        for h in range(3):
            nc.tensor.matmul(ps_oT[:], sb_vm[:, 96 * h:96 * h + 96], A_T(h),
                             start=(h == 0), stop=(h == 2))
        sb_hT = sbt([96, 8], "sb_hT")
        nc.vector.tensor_add(sb_hT[:], ps_oT[:], sb_qkT[:])

        alpha_mms([1])

        # ---- fc1 residual (transposed orientation) ----
        ps_rT = pst([96, 8], "ps_rT")
        nc.tensor.matmul(ps_rT[:], sb_96[:, C_F1:C_F1 + 96], sb_hT[:])
        sb_rT = sbt([96, 8], "sb_rT")
        nc.scalar.activation(sb_rT[:], ps_rT[:], ACT.Relu)
        sb_h2T = sbt([96, 8], "sb_h2T")
        nc.vector.tensor_add(sb_h2T[:], sb_hT[:], sb_rT[:])

        alpha_mms([2])

        # de on the vector engine: (128,4,8) * gate -> reduce over SEQ
        sb_de = sbt([128, 4], "sb_de")
        sb_dp = sbt([128, 32], "sb_dp")
        de_v = sb_t[:, C_DE:C_DE + 32].rearrange("p (m s) -> p m s", m=4)
        g_ap = sb_t[:, C_G:C_G + 8]
        g_bc = bass.AP(tensor=g_ap.tensor, offset=g_ap.offset,
                       ap=[g_ap.ap[0], [0, 4], g_ap.ap[1]])
        nc.vector.tensor_tensor(sb_dp[:].rearrange("p (m s) -> p m s", m=4),
                                de_v, g_bc, ALU.mult)
        nc.vector.tensor_reduce(sb_de[:],
                                sb_dp[:].rearrange("p (m s) -> p m s", m=4),
                                AX.X, ALU.add)

        # ---- stage 2: task query attends to the set (16 heads, d=6) ----
        sb_qT = sbt([96, 1], "sb_qT")
        nc.scalar.activation(sb_qT[:], ps_qT[:], ACT.Relu,
                             bias=sb_96[:, C_BQR:C_BQR + 1])
        ps_k2T = pst([96, 8], "ps_k2T")
        nc.tensor.matmul(ps_k2T[:], sb_96[:, C_WK2:C_WK2 + 96], sb_h2T[:])
        ps_v2T = pst([96, 8], "ps_v2T")
        nc.tensor.matmul(ps_v2T[:], sb_96[:, C_WV2:C_WV2 + 96], sb_h2T[:])
        ps_qqT = pst([96, 1], "ps_qqT")
        nc.tensor.matmul(ps_qqT[:], sb_96[:, C_WQ2:C_WQ2 + 96], sb_qT[:])
        sb_qqT = sbt([96, 1], "sb_qqT")
        nc.scalar.copy(sb_qqT[:], ps_qqT[:])
        sb_v2T = sbt([96, 8], "sb_v2T")
        nc.scalar.copy(sb_v2T[:], ps_v2T[:])
        sb_tmp = sbt([96, 8], "sb_tmp")
        nc.scalar.mul(sb_tmp[:], ps_k2T[:], sb_qqT[:])

        alpha_mms([3])

        ps_s2 = pst([16, 8], "ps_s2")
        nc.tensor.matmul(ps_s2[:], sb_96[:, C_E:C_E + 16], sb_tmp[:])
        sb_e2 = sbt([16, 8], "sb_e2")
        nc.scalar.activation(sb_e2[:], ps_s2[:], ACT.Exp, scale=SCL)
        sb_sum2 = sbt([16, 1], "sb_sum2")
        nc.vector.tensor_reduce(sb_sum2[:], sb_e2[:], AX.X, ALU.add)
        sb_rec2 = sbt([16, 1], "sb_rec2")
        nc.vector.reciprocal(sb_rec2[:], sb_sum2[:])
        sb_a2 = sbt([16, 8], "sb_a2")
        nc.scalar.mul(sb_a2[:], sb_e2[:], sb_rec2[:])

        # alpha sigmoid tail: 1/(1+exp(-(z+ba))) in (128,4)
        sb_zb = sbt([128, 4], "sb_zb")
        nc.vector.tensor_add(sb_zb[:], ps_al[:], sb_t[:, C_BA:C_BA + 4])
        sb_en = sbt([128, 4], "sb_en")
        nc.scalar.activation(sb_en[:], sb_zb[:], ACT.Exp, scale=-1.0)
        sb_dn = sbt([128, 4], "sb_dn")
        nc.vector.tensor_scalar_add(sb_dn[:], sb_en[:], 1.0)
        sb_alp = sbt([128, 4], "sb_alp")
        nc.vector.reciprocal(sb_alp[:], sb_dn[:])

        ps_a2e = pst([96, 8], "ps_a2e")
        nc.tensor.matmul(ps_a2e[:], sb_16[:], sb_a2[:])
        sb_scr = sbt([96, 8], "sb_scr")
        nc.vector.tensor_mul(sb_scr[:], ps_a2e[:], sb_v2T[:])
        sb_o2T = sbt([96, 1], "sb_o2T")
        nc.vector.tensor_reduce(sb_o2T[:], sb_scr[:], AX.X, ALU.add)
        sb_ot1 = sbt([96, 1], "sb_ot1")
        nc.vector.tensor_add(sb_ot1[:], sb_o2T[:], sb_qqT[:])
        ps_r2 = pst([96, 1], "ps_r2")
        nc.tensor.matmul(ps_r2[:], sb_96[:, C_F2:C_F2 + 96], sb_ot1[:])
        sb_r2 = sbt([96, 1], "sb_r2")
        nc.scalar.activation(sb_r2[:], ps_r2[:], ACT.Relu)
        sb_otf = sbt([96, 1], "sb_otf")
        nc.vector.tensor_add(sb_otf[:], sb_ot1[:], sb_r2[:])

        # ---- tail: transT then FiLM mix, all (128,4) ----
        ps_tr = pst([128, 4], "ps_tr")
        for m in range(4):
            nc.tensor.matmul(ps_tr[:, m:m + 1],
                             sb_96[:, C_WO + 128 * m:C_WO + 128 * m + 128],
                             sb_otf[:])
        sb_d1 = sbt([128, 4], "sb_d1")
        nc.vector.tensor_sub(sb_d1[:], ps_tr[:], sb_de[:])
        sb_d2 = sbt([128, 4], "sb_d2")
        nc.vector.tensor_mul(sb_d2[:], sb_d1[:], sb_alp[:])
        sb_mx = sbt([128, 4], "sb_mx")
        nc.vector.tensor_add(sb_mx[:], sb_d2[:], sb_de[:])
        sb_sc = sbt([128, 4], "sb_sc")
        nc.vector.tensor_mul(sb_sc[:], sb_mx[:], sb_t[:, C_RG:C_RG + 4])
        sb_o = sbt([128, 4], "sb_o")
        nc.vector.tensor_scalar_add(sb_o[:], sb_sc[:], sb_t[:, C_OF:C_OF + 1])

        nc.scalar.dma_start(out=d_out[:], in_=sb_o[:])

    nc.compile()
    return nc


def _to_chunks128(a, cols):
    """(512, cols) -> (128, 4*cols) with column block k = rows [128k, 128k+128)."""
    return np.ascontiguousarray(
        a.reshape(4, 128, cols).transpose(1, 0, 2).reshape(128, 4 * cols),
        dtype=np.float32)


def _pack_inputs(inputs):
    gate = np.asarray(inputs['gate'], np.float32)
    x = np.asarray(inputs['x'], np.float32)
    Wa = np.asarray(inputs['Wa'], np.float32)
    ba = np.asarray(inputs['ba'], np.float32)
    Wqr = np.asarray(inputs['Wqr'], np.float32)
    bqr = np.asarray(inputs['bqr'], np.float32)
    P = np.asarray(inputs['P'], np.float32)
    De = np.asarray(inputs['De'], np.float32)
    regs = np.asarray(inputs['regs'], np.float32)

    import ml_dtypes
    wa_p = np.ascontiguousarray(_to_chunks128(Wa, 512).astype(ml_dtypes.bfloat16))
    wqr_p = _to_chunks128(Wqr, 96)
    xT4 = np.ascontiguousarray(x.reshape(4, 128).T, dtype=np.float32)
    baT4 = np.ascontiguousarray(ba.reshape(4, 128).T, dtype=np.float32)
    g128 = np.ascontiguousarray(np.tile(gate.reshape(1, 8), (128, 1)))

    E = np.zeros((96, 16), np.float32)
    E[np.arange(96), np.arange(96) // 6] = 1.0
    b16 = np.ascontiguousarray(E.T)

    in_maps = []
    for i in range(NPROC):
        b, t = i // 4, i % 4
        offs = 1.0 if t in (0, 2) else 0.0
        px = np.concatenate([
            _to_chunks128(P[b, t], 8),
            xT4,
            baT4,
            np.ascontiguousarray(regs[b, t].reshape(4, 128).T),
            np.full((128, 1), offs, np.float32),
            _to_chunks128(De[b, t], 8),
            g128,
        ], axis=1)
        wq1 = np.asarray(inputs['Wq1'], np.float32)[i]
        wk1 = np.asarray(inputs['Wk1'], np.float32)[i]
        wv1 = np.asarray(inputs['Wv1'], np.float32)[i]
        # w1 chunk-major: block k = [wq1_k | wk1_k | wv1_k], each (128, 96)
        w1 = np.concatenate(
            [np.concatenate([wq1[128 * k:128 * k + 128],
                             wk1[128 * k:128 * k + 128],
                             wv1[128 * k:128 * k + 128]], axis=1)
             for k in range(4)], axis=1)
        b96 = np.concatenate([
            np.asarray(inputs['fc1'], np.float32)[i],
            np.asarray(inputs['Wq2'], np.float32)[i],
            np.asarray(inputs['Wk2'], np.float32)[i],
            np.asarray(inputs['Wv2'], np.float32)[i],
            np.asarray(inputs['fc2'], np.float32)[i],
            np.asarray(inputs['Wo'], np.float32)[i],
            E,
            bqr.reshape(96, 1),
        ], axis=1)
        in_maps.append({
            'px': np.ascontiguousarray(px),
            'w1': np.ascontiguousarray(w1),
            'b96': np.ascontiguousarray(b96),
            'wqr': wqr_p,
            'b16': b16,
            'wa': wa_p,
        })
    return in_maps


def _run(inputs, trace=False):
    from concourse.bass_utils import run_bass_kernel_spmd
    if 'nc' not in _CACHE:
        _CACHE['nc'] = _build_nc()
    nc = _CACHE['nc']
    in_maps = _pack_inputs(inputs)
    res = run_bass_kernel_spmd(nc, in_maps, list(range(NPROC)), trace=trace)
    out = np.zeros((NB, 4, NM), np.float32)
    for i in range(NPROC):
        out[i // 4, i % 4] = np.asarray(res.results[i]['out']).T.reshape(NM)
    return out, res


def kernel(**inputs):
    out, _ = _run(inputs, trace=False)
    return out



# revision 6
# speedup vs baseline: 1.4663x; 1.4663x over previous
"""Trainium2 Bass kernel for nn_FilmLayerNetwork.

Sharding: one NeuronCore per SMAB processor (NPROC = 8 = n_cores).
Each core computes its processor's full 512-map output slice.

v2 design notes (from NTFF traces of the 34.5us baseline):
- ALL weights are fp8e4m3 (tolerance is 2e-2; measured e2e error ~1e-4).
  fp8/bf16 matmuls are single-pass (~27-60ns) vs fp32 LOW/HIGH pairs
  (~160-480ns each) and LDWEIGHTS drops ~5x. DMA bytes drop ~3x.
- Four input DMAs (a: P+w1+xT, b: wa+wqr, c: fc1/wq2/wk2/wv2/fc2/wo/E,
  f: fp32 film tail data), issued from four different engines so the
  ~600ns DGE-issue sequencer costs parallelize; transfers still
  serialize on the shared DMA-engine pool in issue order a,f,b,c
  (consumption order).
- The exp activation table load (1.28us) is triggered by a dummy exp at
  program start so it overlaps the input DMA instead of sitting in the
  softmax critical path.
- Intermediates are bf16: single-pass matmuls everywhere, 2x DVE.
- Fusions: relu+residual-add via scalar_tensor_tensor(max,add);
  exp+row-sum via activation(accum_out); (a2e*v2T, sum) via
  tensor_tensor_reduce; FiLM tail algebra reduced to 2 on-path ops
  (out = (alpha*regs)*trans + B) with B = A - alpha*(A - offs),
  A = sum((De*regs | offs) * (gate|1)) precomputed off-path on gpsimd.
- Stage-1 O^T accumulates per-head via tile_position=(0,32h) matmuls
  writing disjoint PSUM partition ranges (no masked-V copies).
"""

import numpy as np

NM, ZG, HID, SEQ = 512, 512, 96, 8
H1, H2, NPROC, NB = 3, 16, 8, 2
SCL = float(1.0 / np.sqrt(96.0))

# a columns: P4 | w1 (4 x [wq1|wk1|wv1]) | xT4
A_P, A_W1, A_XT = 0, 32, 1184
A_COLS = 1188
# b columns: wa (4 x 512) | wqr (4 x 96)
B_WA, B_WQR = 0, 2048
B_COLS = 2432
# c columns (rows 0-95): fc1 | wq2 | wk2 | wv2 | fc2 | wo (4 x 128) |
#   E2T (rows 0-15) | E96
C_F1, C_WQ2, C_WK2, C_WV2, C_F2, C_WO, C_E2T, C_E96 = (
    0, 96, 192, 288, 384, 480, 992, 1088)
C_COLS = 1104
# f columns (fp32): De'9 (4m x 9s) | gate9 | baT4 | bqr | -offs | regsT4
F_DE, F_G9, F_BA, F_BQR, F_NOF, F_RG = 0, 36, 45, 49, 50, 51
F_COLS = 55

_CACHE = {}


def _build_nc():
    import concourse.bass as bass
    import concourse.bacc as bacc
    import concourse.tile as tile
    import concourse.mybir as mybir

    f32 = mybir.dt.float32
    bf16 = mybir.dt.bfloat16
    f8 = mybir.dt.float8e4
    AX = mybir.AxisListType
    ALU = mybir.AluOpType
    ACT = mybir.ActivationFunctionType

    nc = bacc.Bacc("TRN2", target_bir_lowering=False, debug=False,
                   num_devices=NPROC)

    d_a = nc.dram_tensor("a", [128, A_COLS], f8, kind="ExternalInput").ap()
    d_b = nc.dram_tensor("b", [128, B_COLS], f8, kind="ExternalInput").ap()
    d_c = nc.dram_tensor("c", [128, C_COLS], f8, kind="ExternalInput").ap()
    d_f = nc.dram_tensor("f", [128, F_COLS], f32, kind="ExternalInput").ap()
    d_out = nc.dram_tensor("out", [128, 4], f32, kind="ExternalOutput").ap()

    with tile.TileContext(nc) as tc, \
         tc.tile_pool(name="sb", bufs=1) as sb, \
         tc.tile_pool(name="ps", bufs=8, space="PSUM") as ps:

        def sbt(shape, tag, dt=f32):
            return sb.tile(shape, dt, tag=tag, name=tag)

        def pst(shape, tag):
            return ps.tile(shape, f32, tag="ps_shared", name=tag)

        # ---- input DMAs: four transfers, three issuing engines (only
        # SP/ACT have HWDGE; gpsimd uses SWDGE) ----
        sb_a = sbt([128, A_COLS], "sb_a", f8)
        nc.sync.dma_start(out=sb_a[:], in_=d_a[:])
        sb_b = sbt([128, B_COLS], "sb_b", f8)
        nc.scalar.dma_start(out=sb_b[:], in_=d_b[:])
        sb_f = sbt([128, F_COLS], "sb_f")
        nc.gpsimd.dma_start(out=sb_f[:], in_=d_f[:])
        sb_c = sbt([128, C_COLS], "sb_c", f8)
        nc.gpsimd.dma_start(out=sb_c[:], in_=d_c[:])

        # early memsets + dummy exp to pull the ACT table load off the
        # critical path (overlaps the input DMAs)
        sb_z1 = sbt([1, 1], "sb_z1")
        nc.gpsimd.memset(sb_z1[:], 0.0)
        sb_t32 = sbt([32, 288], "sb_t32", bf16)
        nc.gpsimd.memset(sb_t32[:], 0.0)
        sb_a32 = sbt([32, 96], "sb_a32", bf16)
        nc.gpsimd.memset(sb_a32[:], 0.0)
        sb_z1e = sbt([1, 1], "sb_z1e")
        nc.scalar.activation(sb_z1e[:], sb_z1[:], ACT.Exp)

        P_blk = lambda k: sb_a[:, A_P + 8 * k:A_P + 8 * k + 8]
        w1_blk = lambda k: sb_a[:, A_W1 + 288 * k:A_W1 + 288 * k + 288]
        xT_blk = lambda k: sb_a[:, A_XT + k:A_XT + k + 1]

        # ---- stage 0: [Qk | Kk | Vv] (8, 288) in 4 fused fp8 matmuls ----
        ps_qkv = pst([8, 288], "ps_qkv")
        for k in range(4):
            nc.tensor.matmul(ps_qkv[:], P_blk(k), w1_blk(k),
                             start=(k == 0), stop=(k == 3))

        # one copy into the 32-col-block transpose layout (bf16);
        # Vv stays untransposed at cols 192:288
        nc.scalar.copy(sb_t32[0:8, 0:288], ps_qkv[:])

        sb_tT = sbt([32, 192], "sb_tT", bf16)
        nc.vector.transpose(sb_tT[:], sb_t32[:, 0:192])

        def QkT_h(h):
            return sb_tT[0:32, 32 * h:32 * h + 8]

        def KkT_h(h):
            return sb_tT[0:32, 96 + 32 * h:96 + 32 * h + 8]

        def Vv_h(h):
            return sb_t32[0:8, 192 + 32 * h:192 + 32 * h + 32]

        # MHA1 scores, per head
        ps_s = pst([8, 24], "ps_s")
        for h in range(3):
            nc.tensor.matmul(ps_s[:, 8 * h:8 * h + 8], QkT_h(h), KkT_h(h))

        # qT: 4 fp8 contraction chunks over Wqr (b landed by now)
        ps_qT = pst([96, 1], "ps_qT")
        for k in range(4):
            nc.tensor.matmul(ps_qT[:],
                             sb_b[:, B_WQR + 96 * k:B_WQR + 96 * k + 96],
                             xT_blk(k), start=(k == 0), stop=(k == 3))

        # alphaT (128,4): 16 fp8 (k,m) chunk matmuls, interleaved into the
        # chain's PE gaps
        ps_al = pst([128, 4], "ps_al")

        def alpha_mms(ms):
            for m in ms:
                for k in range(4):
                    nc.tensor.matmul(
                        ps_al[:, m:m + 1],
                        sb_b[:, 512 * k + 128 * m:512 * k + 128 * m + 128],
                        xT_blk(k), start=(k == 0), stop=(k == 3))

        alpha_mms([0, 1])

        # QkT (96,8) contiguous for the attention residual (ACT copies can
        # shift partitions; runs during softmax)
        sb_qkT = sbt([96, 8], "sb_qkT", bf16)
        for h in range(3):
            nc.scalar.copy(sb_qkT[32 * h:32 * h + 32, :], QkT_h(h))

        # softmax1 (no max-subtraction; magnitudes are small), normalized A
        # written into the 32x32-block layout
        a32v = sb_a32[0:8, :].rearrange("p (h x) -> p h x", h=3)[:, :, 0:8]
        nc.scalar.activation(a32v, ps_s[:].rearrange("p (h x) -> p h x", h=3),
                             ACT.Exp, scale=SCL)
        sb_sums = sbt([8, 3], "sb_sums")
        nc.vector.tensor_reduce(sb_sums[:], a32v, AX.X, ALU.add)
        sb_rec = sbt([8, 3], "sb_rec")
        nc.vector.reciprocal(sb_rec[:], sb_sums[:])
        rec_ap = sb_rec[:]
        rec_bc = bass.AP(tensor=rec_ap.tensor, offset=rec_ap.offset,
                         ap=[rec_ap.ap[0], rec_ap.ap[1], [0, 8]])
        nc.vector.tensor_tensor(a32v, a32v, rec_bc, ALU.mult)
        sb_aT32 = sbt([32, 96], "sb_aT32", bf16)
        nc.vector.transpose(sb_aT32[:], sb_a32[:])

        def A_T(h):
            return sb_aT32[0:8, 32 * h:32 * h + 8]

        # O^T per head via tile_position: each head writes its own 32-row
        # PSUM partition range
        ps_oT = pst([96, 8], "ps_oT")
        for h in range(3):
            nc.tensor.matmul(ps_oT[32 * h:32 * h + 32, :], Vv_h(h), A_T(h),
                             tile_position=(0, 32 * h))

        # qqT = wq2^T @ qT (qT relu'd on ACT during softmax)
        sb_qT = sbt([96, 1], "sb_qT", bf16)
        nc.scalar.activation(sb_qT[:], ps_qT[:], ACT.Relu,
                             bias=sb_f[0:96, F_BQR:F_BQR + 1])
        ps_qqT = pst([96, 1], "ps_qqT")
        nc.tensor.matmul(ps_qqT[:], sb_c[0:96, C_WQ2:C_WQ2 + 96], sb_qT[:])

        alpha_mms([2, 3])

        sb_hT = sbt([96, 8], "sb_hT", bf16)
        nc.vector.tensor_add(sb_hT[:], ps_oT[:], sb_qkT[:])

        # ---- fc1 residual: h2T = hT + relu(fc1^T @ hT), fused ----
        ps_rT = pst([96, 8], "ps_rT")
        nc.tensor.matmul(ps_rT[:], sb_c[0:96, C_F1:C_F1 + 96], sb_hT[:])
        sb_h2T = sbt([96, 8], "sb_h2T", bf16)
        nc.vector.scalar_tensor_tensor(sb_h2T[:], ps_rT[:], 0.0, sb_hT[:],
                                       ALU.max, ALU.add)

        # de/A precompute on gpsimd (only needs f): A = sum(De'9 * gate9)
        # with regs and the +offs fold baked in host-side
        sb_dp = sbt([128, 36], "sb_dp")
        de_v = sb_f[:, F_DE:F_DE + 36].rearrange("p (m s) -> p m s", m=4)
        g_ap = sb_f[:, F_G9:F_G9 + 9]
        g_bc = bass.AP(tensor=g_ap.tensor, offset=g_ap.offset,
                       ap=[g_ap.ap[0], [0, 4], g_ap.ap[1]])
        nc.gpsimd.tensor_tensor(sb_dp[:].rearrange("p (m s) -> p m s", m=4),
                                de_v, g_bc, ALU.mult)
        sb_A = sbt([128, 4], "sb_A")
        nc.vector.tensor_reduce(sb_A[:],
                                sb_dp[:].rearrange("p (m s) -> p m s", m=4),
                                AX.X, ALU.add)
        # de_r = A - offs  (F_NOF holds -offs)
        sb_der = sbt([128, 4], "sb_der")
        nc.gpsimd.tensor_scalar_add(sb_der[:], sb_A[:],
                                    sb_f[:, F_NOF:F_NOF + 1])

        # ---- stage 2 ----
        ps_k2T = pst([96, 8], "ps_k2T")
        nc.tensor.matmul(ps_k2T[:], sb_c[0:96, C_WK2:C_WK2 + 96], sb_h2T[:])
        ps_v2T = pst([96, 8], "ps_v2T")
        nc.tensor.matmul(ps_v2T[:], sb_c[0:96, C_WV2:C_WV2 + 96], sb_h2T[:])

        sb_qqT = sbt([96, 1], "sb_qqT")
        nc.scalar.copy(sb_qqT[:], ps_qqT[:])
        sb_tmp = sbt([96, 8], "sb_tmp", bf16)
        nc.scalar.mul(sb_tmp[:], ps_k2T[:], sb_qqT[:])

        ps_s2 = pst([16, 8], "ps_s2")
        nc.tensor.matmul(ps_s2[:], sb_c[0:96, C_E96:C_E96 + 16], sb_tmp[:])

        # alpha sigmoid tail: 1/(1+exp(-(z+ba))) in (128,4)
        sb_zb = sbt([128, 4], "sb_zb")
        nc.vector.tensor_add(sb_zb[:], ps_al[:], sb_f[:, F_BA:F_BA + 4])
        sb_en = sbt([128, 4], "sb_en")
        nc.scalar.activation(sb_en[:], sb_zb[:], ACT.Exp, scale=-1.0)

        sb_v2T = sbt([96, 8], "sb_v2T", bf16)
        nc.scalar.copy(sb_v2T[:], ps_v2T[:])

        # softmax2: exp with fused row-sum accumulator
        sb_e2 = sbt([16, 8], "sb_e2", bf16)
        sb_sum2 = sbt([16, 1], "sb_sum2")
        nc.scalar.activation(sb_e2[:], ps_s2[:], ACT.Exp, scale=SCL,
                             accum_out=sb_sum2[:])
        sb_rec2 = sbt([16, 1], "sb_rec2")
        nc.vector.reciprocal(sb_rec2[:], sb_sum2[:])
        sb_a2 = sbt([16, 8], "sb_a2", bf16)
        nc.scalar.mul(sb_a2[:], sb_e2[:], sb_rec2[:])

        ps_a2e = pst([96, 8], "ps_a2e")
        nc.tensor.matmul(ps_a2e[:], sb_c[0:16, C_E2T:C_E2T + 96], sb_a2[:])

        # alpha tail on DVE; alr/B on gpsimd once alpha is ready
        sb_dn = sbt([128, 4], "sb_dn")
        nc.vector.tensor_scalar_add(sb_dn[:], sb_en[:], 1.0)
        sb_alp = sbt([128, 4], "sb_alp")
        nc.vector.reciprocal(sb_alp[:], sb_dn[:])
        sb_alr = sbt([128, 4], "sb_alr")
        nc.gpsimd.tensor_tensor(sb_alr[:], sb_alp[:],
                                sb_f[:, F_RG:F_RG + 4], ALU.mult)
        sb_D = sbt([128, 4], "sb_D")
        nc.gpsimd.tensor_tensor(sb_D[:], sb_alp[:], sb_der[:], ALU.mult)
        sb_B = sbt([128, 4], "sb_B")
        nc.gpsimd.tensor_tensor(sb_B[:], sb_A[:], sb_D[:], ALU.subtract)

        # O2 = sum_h A2 * V2 (broadcast via E2T matmul)
        # (tensor_tensor_reduce crashes HW - NRT_EXEC_UNIT_UNRECOVERABLE)
        sb_scr = sbt([96, 8], "sb_scr")
        sb_o2T = sbt([96, 1], "sb_o2T")
        nc.vector.tensor_mul(sb_scr[:], ps_a2e[:], sb_v2T[:])
        nc.vector.tensor_reduce(sb_o2T[:], sb_scr[:], AX.X, ALU.add)
        sb_ot1 = sbt([96, 1], "sb_ot1", bf16)
        nc.vector.tensor_add(sb_ot1[:], sb_o2T[:], sb_qqT[:])

        # fc2 residual, fused relu+add
        ps_r2 = pst([96, 1], "ps_r2")
        nc.tensor.matmul(ps_r2[:], sb_c[0:96, C_F2:C_F2 + 96], sb_ot1[:])
        sb_otf = sbt([96, 1], "sb_otf", bf16)
        nc.vector.scalar_tensor_tensor(sb_otf[:], ps_r2[:], 0.0, sb_ot1[:],
                                       ALU.max, ALU.add)

        # ---- tail: transT then 2-op FiLM mix ----
        ps_tr = pst([128, 4], "ps_tr")
        for m in range(4):
            nc.tensor.matmul(ps_tr[:, m:m + 1],
                             sb_c[0:96, C_WO + 128 * m:C_WO + 128 * m + 128],
                             sb_otf[:])
        sb_t1 = sbt([128, 4], "sb_t1")
        nc.vector.tensor_mul(sb_t1[:], ps_tr[:], sb_alr[:])
        sb_o = sbt([128, 4], "sb_o")
        nc.vector.tensor_add(sb_o[:], sb_t1[:], sb_B[:])

        nc.scalar.dma_start(out=d_out[:], in_=sb_o[:])

    nc.compile()
    return nc


def _to_chunks128(a, cols):
    """(512, cols) -> (128, 4*cols) with column block k = rows [128k, 128k+128)."""
    return np.ascontiguousarray(
        a.reshape(4, 128, cols).transpose(1, 0, 2).reshape(128, 4 * cols))


def _pack_inputs(inputs):
    import ml_dtypes
    fp8 = ml_dtypes.float8_e4m3
    f32 = np.float32

    gate = np.asarray(inputs['gate'], f32)
    x = np.asarray(inputs['x'], f32)
    Wa = np.asarray(inputs['Wa'], f32)
    ba = np.asarray(inputs['ba'], f32)
    Wqr = np.asarray(inputs['Wqr'], f32)
    bqr = np.asarray(inputs['bqr'], f32)
    P = np.asarray(inputs['P'], f32)
    De = np.asarray(inputs['De'], f32)
    regs = np.asarray(inputs['regs'], f32)
    Wq1 = np.asarray(inputs['Wq1'], f32)
    Wk1 = np.asarray(inputs['Wk1'], f32)
    Wv1 = np.asarray(inputs['Wv1'], f32)
    fc1 = np.asarray(inputs['fc1'], f32)
    Wq2 = np.asarray(inputs['Wq2'], f32)
    Wk2 = np.asarray(inputs['Wk2'], f32)
    Wv2 = np.asarray(inputs['Wv2'], f32)
    fc2 = np.asarray(inputs['fc2'], f32)
    Wo = np.asarray(inputs['Wo'], f32)

    # b: wa chunks then wqr chunks (shared across cores)
    b_pack = np.concatenate([
        _to_chunks128(Wa, NM),
        _to_chunks128(Wqr, HID),
    ], axis=1).astype(fp8)
    b_pack = np.ascontiguousarray(b_pack)

    xT4 = np.ascontiguousarray(x.reshape(4, 128).T)
    baT4 = np.ascontiguousarray(ba.reshape(4, 128).T)
    g9 = np.concatenate([gate.reshape(1, 8), [[1.0]]], axis=1)
    g9_128 = np.ascontiguousarray(np.tile(g9, (128, 1)), f32)

    # E masks for 16 heads of dim 6
    idx = np.arange(HID) // 6
    E96 = np.zeros((96, 16), f32)
    E96[np.arange(96), idx] = 1.0
    E2T = np.ascontiguousarray(E96.T)         # (16, 96)

    in_maps = []
    for i in range(NPROC):
        bi, t = i // 4, i % 4
        offs = 1.0 if t in (0, 2) else 0.0

        w1 = np.concatenate(
            [np.concatenate([Wq1[i][128 * k:128 * k + 128],
                             Wk1[i][128 * k:128 * k + 128],
                             Wv1[i][128 * k:128 * k + 128]], axis=1)
             for k in range(4)], axis=1)
        a_pack = np.ascontiguousarray(np.concatenate([
            _to_chunks128(P[bi, t], SEQ), w1, xT4], axis=1).astype(fp8))

        c_np = np.zeros((128, C_COLS), f32)
        c_np[0:96, C_F1:C_F1 + 96] = fc1[i]
        c_np[0:96, C_WQ2:C_WQ2 + 96] = Wq2[i]
        c_np[0:96, C_WK2:C_WK2 + 96] = Wk2[i]
        c_np[0:96, C_WV2:C_WV2 + 96] = Wv2[i]
        c_np[0:96, C_F2:C_F2 + 96] = fc2[i]
        c_np[0:96, C_WO:C_WO + 512] = Wo[i]
        c_np[0:16, C_E2T:C_E2T + 96] = E2T
        c_np[0:96, C_E96:C_E96 + 16] = E96
        c_pack = np.ascontiguousarray(c_np.astype(fp8))

        de9 = np.zeros((128, 4, 9), f32)
        de9[:, :, 0:8] = (De[bi, t] * regs[bi, t][:, None]).reshape(
            4, 128, SEQ).transpose(1, 0, 2)
        de9[:, :, 8] = offs
        f_np = np.zeros((128, F_COLS), f32)
        f_np[:, F_DE:F_DE + 36] = de9.reshape(128, 36)
        f_np[:, F_G9:F_G9 + 9] = g9_128
        f_np[:, F_BA:F_BA + 4] = baT4
        f_np[0:96, F_BQR] = bqr
        f_np[:, F_NOF] = -offs
        f_np[:, F_RG:F_RG + 4] = regs[bi, t].reshape(4, 128).T

        in_maps.append({
            'a': a_pack,
            'b': b_pack,
            'c': c_pack,
            'f': np.ascontiguousarray(f_np),
        })
    return in_maps


def _run(inputs, trace=False):
    from concourse.bass_utils import run_bass_kernel_spmd
    if 'nc' not in _CACHE:
        _CACHE['nc'] = _build_nc()
    nc = _CACHE['nc']
    in_maps = _pack_inputs(inputs)
    res = run_bass_kernel_spmd(nc, in_maps, list(range(NPROC)), trace=trace)
    out = np.zeros((NB, 4, NM), np.float32)
    for i in range(NPROC):
        out[i // 4, i % 4] = np.asarray(res.results[i]['out']).T.reshape(NM)
    return out, res


def kernel(**inputs):
    out, _ = _run(inputs, trace=False)
    return out
